# revision 8
# baseline (speedup 1.0000x reference)
"""ASK loss (soft nearest-neighbor NLL) on 8 Trainium2 NeuronCores.

Math (matches the jax reference exactly):
    dist[m,n]  = sqrt(||x_m||^2 + ||r_n||^2 - 2 x_m.r_n)
    score      = softmax(-dist, axis=n)
    soft_nns   = segment_sum(score over classes of y_ref) + EPS
    loss       = -mean_m log(soft_nns[m, y[m]])

Because the softmax normalizer cancels in the ratio, we compute with
unnormalized E = exp(-dist):
    S[c,m] = sum_{n: y_ref[n]=c} E[m,n],   Z[m] = sum_n E[m,n]
    loss_m = log(S[y_m,m] + EPS*Z[m]) - log(Z[m]);  loss = -mean loss_m

Sharding: data-parallel over the batch M. Each core owns 512 rows of x and
streams the full (pre-transposed, -2-scaled) reference set.

Per-core device pipeline, per 128-reference block b (256 blocks):
    PE : psum[n,m] = sum_kc (-2 ref^T chunk).T @ x^T chunk   (4 fp32 matmuls)
    DVE: v = (psum + r2[n]) + x2bcast[m]                     (1 fused STT op)
    ACT: v -> ln -> exp(0.5.) -> exp(-1.) == exp(-sqrt(v))   (wide, in-place,
         single act table set: natural_log_exp_and_others)
    PE : S[0:11, m] += onehot(y_ref)^T @ E                   (psum accumulate)
Epilogue picks S[y_m, m] via a one-hot of y, takes logs on ACT, and DMAs
per-row log-probs out; the host averages 8x512 values.
"""

import numpy as np

M, N, D = 4096, 32768, 512
NCORES = 8
ML = M // NCORES          # 512 rows of x per core
NB = N // 128             # 256 reference blocks
KC = D // 128             # 4 contraction chunks
NCLS = 10
EPS = 1e-6
W = 8                     # blocks fused into one wide ACT group
IW = NCLS + 1             # indicator cols per block: col 0 = ones (Z), 1+c = class c

_CACHE = {}


def _patch_act_tables():
    """Restrict Ln/Exp membership to natural_log_exp_and_others so bacc's
    greedy table chooser emits exactly one ACT table load for the kernel.
    Entry order (= act_func_set_id) is preserved; the actually-loaded set
    genuinely contains both functions."""
    import concourse.bacc as bacc_mod
    import concourse.hw_specs as hw_specs
    import concourse.mybir as mybir

    real = hw_specs.get_activation_tables

    def patched(arch):
        tabs = dict(real(arch))
        ln = mybir.ActivationFunctionType.Ln
        ex = mybir.ActivationFunctionType.Exp
        out = {}
        for name, fns in tabs.items():
            if name != "natural_log_exp_and_others":
                fns = fns - {ln, ex}
            out[name] = fns
        return out

    bacc_mod.get_activation_tables = patched


def _build():
    import concourse.bass as bass
    import concourse.bacc as bacc
    import concourse.mybir as mybir
    import concourse.tile as tile

    _patch_act_tables()

    f32 = mybir.dt.float32
    AF = mybir.ActivationFunctionType
    ADD = mybir.AluOpType.add
    MUL = mybir.AluOpType.mult
    SUB = mybir.AluOpType.subtract

    nc = bacc.Bacc("TRN2", target_bir_lowering=False, debug=False)

    reft = nc.dram_tensor("reft", [NB, 128, D], f32, kind="ExternalInput").ap()
    xt = nc.dram_tensor("xt", [128, KC * ML], f32, kind="ExternalInput").ap()
    x2b = nc.dram_tensor("x2b", [128, ML], f32, kind="ExternalInput").ap()
    r2t = nc.dram_tensor("r2t", [128, NB], f32, kind="ExternalInput").ap()
    ind = nc.dram_tensor("ind", [128, NB * IW], f32, kind="ExternalInput").ap()
    yb = nc.dram_tensor("yb", [IW, ML], f32, kind="ExternalInput").ap()
    out_ld = nc.dram_tensor("out_ld", [1, ML], f32, kind="ExternalOutput").ap()

    with tile.TileContext(nc) as tc:
        with (
            tc.tile_pool(name="const", bufs=1) as constp,
            tc.tile_pool(name="refp", bufs=4) as refp,
            tc.tile_pool(name="wide", bufs=3) as widep,
            tc.tile_pool(name="epi", bufs=1) as epip,
            tc.tile_pool(name="pdot", bufs=4, space=bass.MemorySpace.PSUM) as pdot,
            tc.tile_pool(name="pacc", bufs=1, space=bass.MemorySpace.PSUM) as pacc,
            tc.tile_pool(name="pone", bufs=1, space=bass.MemorySpace.PSUM) as pone,
        ):
            xt_sb = constp.tile([128, KC * ML], f32)
            x2b_sb = constp.tile([128, ML], f32)
            r2_sb = constp.tile([128, NB], f32)
            ind_sb = constp.tile([128, NB * IW], f32)
            yb_sb = constp.tile([IW, ML], f32)
            ones_sb = constp.tile([IW, 1], f32)
            nc.sync.dma_start(xt_sb[:], xt[:])
            nc.sync.dma_start(x2b_sb[:], x2b[:])
            nc.sync.dma_start(r2_sb[:], r2t[:])
            nc.sync.dma_start(ind_sb[:], ind[:])
            nc.sync.dma_start(yb_sb[:], yb[:])
            nc.vector.memset(ones_sb[:], 1.0)

            S = pacc.tile([IW, ML], f32)

            for g in range(NB // W):
                s_w = widep.tile([128, W * ML], f32)
                for w in range(W):
                    b = g * W + w
                    ref_sb = refp.tile([128, D], f32)
                    nc.sync.dma_start(ref_sb[:], reft[b])
                    pd = pdot.tile([128, ML], f32)
                    for kc in range(KC):
                        nc.tensor.matmul(
                            pd[:],
                            ref_sb[:, kc * 128 : (kc + 1) * 128],
                            xt_sb[:, kc * ML : (kc + 1) * ML],
                            start=(kc == 0),
                            stop=(kc == KC - 1),
                        )
                    # v = (dot + r2[n]) + x2[m]  (r2 per-partition, x2 bcast)
                    nc.vector.scalar_tensor_tensor(
                        s_w[:, w * ML : (w + 1) * ML],
                        pd[:],
                        r2_sb[:, b : b + 1],
                        x2b_sb[:],
                        op0=ADD,
                        op1=ADD,
                    )
                # E = exp(-sqrt(v)) = exp(-exp(0.5*ln(v))), in place, wide
                nc.scalar.activation(s_w[:], s_w[:], AF.Ln)
                nc.scalar.activation(s_w[:], s_w[:], AF.Exp, scale=0.5)
                nc.scalar.activation(s_w[:], s_w[:], AF.Exp, scale=-1.0)
                for w in range(W):
                    b = g * W + w
                    nc.tensor.matmul(
                        S[:],
                        ind_sb[:, b * IW : (b + 1) * IW],
                        s_w[:, w * ML : (w + 1) * ML],
                        start=(b == 0),
                        stop=(b == NB - 1),
                    )

            # ---- epilogue: loss_m = log(S[y_m] + EPS*Z) - log(Z) ----
            # S row 0 = Z (ones column of the indicator); rows 1..10 = classes.
            # yb row 0 is zero, so the full-11-partition ops never slice at
            # partition offsets the BIR verifier rejects.
            t_sb = epip.tile([IW, ML], f32)
            nc.vector.tensor_tensor(t_sb[:], S[:], yb_sb[:], MUL)
            p0 = pone.tile([1, ML], f32)
            nc.tensor.matmul(p0[:], ones_sb[:], t_sb[:], start=True, stop=True)
            p0_sb = epip.tile([1, ML], f32)
            nc.scalar.copy(p0_sb[:], p0[:])
            t2_sb = epip.tile([1, ML], f32)
            # t2 = (Z * EPS) + p0
            nc.vector.scalar_tensor_tensor(
                t2_sb[:], S[0:1, :], float(EPS), p0_sb[:], op0=MUL, op1=ADD
            )
            l1_sb = epip.tile([1, ML], f32)
            l2_sb = epip.tile([1, ML], f32)
            nc.scalar.activation(l1_sb[:], t2_sb[:], AF.Ln)
            nc.scalar.activation(l2_sb[:], S[0:1, :], AF.Ln)
            ld_sb = epip.tile([1, ML], f32)
            nc.vector.tensor_tensor(ld_sb[:], l1_sb[:], l2_sb[:], SUB)
            nc.sync.dma_start(out_ld[:], ld_sb[:])

    nc.compile()
    return nc


def _get_nc():
    if "nc" not in _CACHE:
        _CACHE["nc"] = _build()
    return _CACHE["nc"]


def _prep_inputs(x, x_ref, y, y_ref):
    x = np.ascontiguousarray(np.asarray(x, dtype=np.float32))
    x_ref = np.ascontiguousarray(np.asarray(x_ref, dtype=np.float32))
    y = np.asarray(y).astype(np.int64)
    y_ref = np.asarray(y_ref).astype(np.int64)

    # shared across cores ------------------------------------------------
    # reft[b, k, kc*128+n] = -2 * x_ref[b*128+n, kc*128+k]
    r4 = (-2.0 * x_ref).reshape(NB, 128, KC, 128)         # [b, n, kc, k]
    reft = np.ascontiguousarray(
        r4.transpose(0, 3, 2, 1).reshape(NB, 128, D), dtype=np.float32
    )
    r2 = (x_ref.astype(np.float64) ** 2).sum(1).astype(np.float32)  # [N]
    r2t = np.ascontiguousarray(r2.reshape(NB, 128).T)     # [128, NB]
    # ind[p, b*IW+0] = 1 (Z row); ind[p, b*IW+1+c] = 1{y_ref[b*128+p] == c}
    yr = y_ref.reshape(NB, 128)
    indm = np.zeros((NB, 128, IW), dtype=np.float32)
    bi, pi = np.meshgrid(np.arange(NB), np.arange(128), indexing="ij")
    indm[bi, pi, 1 + yr] = 1.0
    indm[:, :, 0] = 1.0
    ind = np.ascontiguousarray(indm.transpose(1, 0, 2).reshape(128, NB * IW))

    in_maps = []
    for c in range(NCORES):
        xc = x[c * ML : (c + 1) * ML]                      # [ML, D]
        # xt[k, kc*ML+m] = xc[m, kc*128+k]
        xt = np.ascontiguousarray(
            xc.reshape(ML, KC, 128).transpose(2, 1, 0).reshape(128, KC * ML)
        )
        x2 = (xc.astype(np.float64) ** 2).sum(1).astype(np.float32)  # [ML]
        x2b = np.ascontiguousarray(np.broadcast_to(x2, (128, ML)))
        yc = y[c * ML : (c + 1) * ML]
        ybm = np.zeros((IW, ML), dtype=np.float32)
        ybm[1 + yc, np.arange(ML)] = 1.0
        in_maps.append(
            {
                "reft": reft,
                "xt": xt,
                "x2b": x2b,
                "r2t": r2t,
                "ind": ind,
                "yb": ybm,
            }
        )
    return in_maps


def run(x, x_ref, y, y_ref, trace=False, trace_kwargs=None):
    from concourse.bass_utils import run_bass_kernel_spmd

    nc = _get_nc()
    in_maps = _prep_inputs(x, x_ref, y, y_ref)
    res = run_bass_kernel_spmd(
        nc,
        in_maps,
        list(range(NCORES)),
        trace=trace,
        **(trace_kwargs or {}),
    )
    ld = np.concatenate([res.results[c]["out_ld"].reshape(-1) for c in range(NCORES)])
    loss = np.float32(-(ld.astype(np.float64).mean()))
    return loss, res


def kernel(x, x_ref, y, y_ref):
    loss, _ = run(x, x_ref, y, y_ref)
    return np.asarray(loss, dtype=np.float32)


# revision 14
# speedup vs baseline: 2.1086x; 2.1086x over previous
"""ASK loss (soft nearest-neighbor NLL) on 8 Trainium2 NeuronCores.

Math (matches the jax reference exactly):
    dist[m,n]  = sqrt(||x_m||^2 + ||r_n||^2 - 2 x_m.r_n)
    score      = softmax(-dist, axis=n)
    soft_nns   = segment_sum(score over classes of y_ref) + EPS
    loss       = -mean_m log(soft_nns[m, y[m]])

Because the softmax normalizer cancels in the ratio, we compute with
unnormalized E = exp(-dist):
    S[c,m] = sum_{n: y_ref[n]=c} E[m,n],   Z[m] = sum_n E[m,n]
    loss_m = log(S[y_m,m] + EPS*Z[m]) - log(Z[m]);  loss = -mean loss_m

Sharding: data-parallel over the batch M. Each core owns 512 rows of x and
streams the full (pre-transposed, -2-scaled) reference set.

Per-core device pipeline, per 128-reference block b (256 blocks):
    PE : psum[n,m] = sum_kc (-2 ref^T chunk).T @ x^T chunk   (8 bf16 matmuls:
         both operands split into bf16 hi+lo and stacked along K, so the
         product keeps ~fp32 accuracy at bf16 stream rate)
    DVE: v = (psum + r2[n]) + x2bcast[m]                     (1 fused STT op)
    ACT: v -> ln -> exp(0.5.) -> exp(-1.) == exp(-sqrt(v))   (wide, in-place,
         single act table set: natural_log_exp_and_others; last exp emits
         bf16 for the reduction matmul)
    PE : S[0:11, m] += onehot(y_ref)^T @ E                   (bf16, psum acc)
Epilogue picks S[y_m, m] via a one-hot of y, takes logs on ACT, and DMAs
per-row log-probs out; the host averages 8x512 values.
"""

import numpy as np

M, N, D = 4096, 32768, 512
NCORES = 8
ML = M // NCORES          # 512 rows of x per core
NB = N // 128             # 256 reference blocks
KC = D // 128             # 4 contraction chunks
NCLS = 10
EPS = 1e-6
W = 8                     # blocks fused into one wide ACT group
IW = NCLS + 1             # indicator cols per block: col 0 = ones (Z), 1+c = class c
KK = 2 * KC               # bf16 hi/lo split doubles the contraction chunks

_CACHE = {}


def _patch_act_tables():
    """Restrict Ln/Exp membership to natural_log_exp_and_others so bacc's
    greedy table chooser emits exactly one ACT table load for the kernel.
    Entry order (= act_func_set_id) is preserved; the actually-loaded set
    genuinely contains both functions."""
    import concourse.bacc as bacc_mod
    import concourse.hw_specs as hw_specs
    import concourse.mybir as mybir

    real = hw_specs.get_activation_tables

    def patched(arch):
        tabs = dict(real(arch))
        ln = mybir.ActivationFunctionType.Ln
        ex = mybir.ActivationFunctionType.Exp
        out = {}
        for name, fns in tabs.items():
            if name != "natural_log_exp_and_others":
                fns = fns - {ln, ex}
            out[name] = fns
        return out

    bacc_mod.get_activation_tables = patched


def _build():
    import concourse.bass as bass
    import concourse.bacc as bacc
    import concourse.mybir as mybir
    import concourse.tile as tile

    _patch_act_tables()

    f32 = mybir.dt.float32
    bf16 = mybir.dt.bfloat16
    AF = mybir.ActivationFunctionType
    ADD = mybir.AluOpType.add
    MUL = mybir.AluOpType.mult
    SUB = mybir.AluOpType.subtract

    nc = bacc.Bacc("TRN2", target_bir_lowering=False, debug=False)

    reft = nc.dram_tensor("reft", [NB, 128, KK * 128], bf16, kind="ExternalInput").ap()
    xt = nc.dram_tensor("xt", [128, KK * ML], bf16, kind="ExternalInput").ap()
    x2b = nc.dram_tensor("x2b", [128, ML], f32, kind="ExternalInput").ap()
    r2t = nc.dram_tensor("r2t", [128, NB], f32, kind="ExternalInput").ap()
    ind = nc.dram_tensor("ind", [128, NB * IW], bf16, kind="ExternalInput").ap()
    yb = nc.dram_tensor("yb", [IW, ML], f32, kind="ExternalInput").ap()
    out_ld = nc.dram_tensor("out_ld", [1, ML], f32, kind="ExternalOutput").ap()

    with tile.TileContext(nc) as tc:
        with (
            tc.tile_pool(name="const", bufs=1) as constp,
            tc.tile_pool(name="refp", bufs=4) as refp,
            tc.tile_pool(name="wide", bufs=3) as widep,
            tc.tile_pool(name="epi", bufs=1) as epip,
            tc.tile_pool(name="pdot", bufs=4, space=bass.MemorySpace.PSUM) as pdot,
            tc.tile_pool(name="pacc", bufs=1, space=bass.MemorySpace.PSUM) as pacc,
            tc.tile_pool(name="pone", bufs=1, space=bass.MemorySpace.PSUM) as pone,
        ):
            xt_sb = constp.tile([128, KK * ML], bf16)
            x2b_sb = constp.tile([128, ML], f32)
            r2_sb = constp.tile([128, NB], f32)
            ind_sb = constp.tile([128, NB * IW], bf16)
            yb_sb = constp.tile([IW, ML], f32)
            ones_sb = constp.tile([IW, 1], f32)
            nc.sync.dma_start(xt_sb[:], xt[:])
            nc.sync.dma_start(x2b_sb[:], x2b[:])
            nc.sync.dma_start(r2_sb[:], r2t[:])
            nc.sync.dma_start(ind_sb[:], ind[:])
            nc.sync.dma_start(yb_sb[:], yb[:])
            nc.vector.memset(ones_sb[:], 1.0)

            S = pacc.tile([IW, ML], f32)

            for g in range(NB // W):
                s_w = widep.tile([128, W * ML], f32)
                e_w = widep.tile([128, W * ML], bf16, tag="e_w")
                for w in range(W):
                    b = g * W + w
                    ref_sb = refp.tile([128, KK * 128], bf16)
                    nc.sync.dma_start(ref_sb[:], reft[b])
                    pd = pdot.tile([128, ML], f32)
                    for kc in range(KK):
                        nc.tensor.matmul(
                            pd[:],
                            ref_sb[:, kc * 128 : (kc + 1) * 128],
                            xt_sb[:, kc * ML : (kc + 1) * ML],
                            start=(kc == 0),
                            stop=(kc == KK - 1),
                        )
                    # v = (dot + r2[n]) + x2[m]  (r2 per-partition, x2 bcast)
                    nc.vector.scalar_tensor_tensor(
                        s_w[:, w * ML : (w + 1) * ML],
                        pd[:],
                        r2_sb[:, b : b + 1],
                        x2b_sb[:],
                        op0=ADD,
                        op1=ADD,
                    )
                # E = exp(-sqrt(v)) = exp(-exp(0.5*ln(v))), wide; last exp
                # downcasts to bf16 for the reduction matmul
                nc.scalar.activation(s_w[:], s_w[:], AF.Ln)
                nc.scalar.activation(s_w[:], s_w[:], AF.Exp, scale=0.5)
                nc.scalar.activation(e_w[:], s_w[:], AF.Exp, scale=-1.0)
                for w in range(W):
                    b = g * W + w
                    nc.tensor.matmul(
                        S[:],
                        ind_sb[:, b * IW : (b + 1) * IW],
                        e_w[:, w * ML : (w + 1) * ML],
                        start=(b == 0),
                        stop=(b == NB - 1),
                    )

            # ---- epilogue: loss_m = log(S[y_m] + EPS*Z) - log(Z) ----
            # S row 0 = Z (ones column of the indicator); rows 1..10 = classes.
            # yb row 0 is zero, so the full-11-partition ops never slice at
            # partition offsets the BIR verifier rejects.
            t_sb = epip.tile([IW, ML], f32)
            nc.vector.tensor_tensor(t_sb[:], S[:], yb_sb[:], MUL)
            p0 = pone.tile([1, ML], f32)
            nc.tensor.matmul(p0[:], ones_sb[:], t_sb[:], start=True, stop=True)
            p0_sb = epip.tile([1, ML], f32)
            nc.scalar.copy(p0_sb[:], p0[:])
            t2_sb = epip.tile([1, ML], f32)
            # t2 = (Z * EPS) + p0
            nc.vector.scalar_tensor_tensor(
                t2_sb[:], S[0:1, :], float(EPS), p0_sb[:], op0=MUL, op1=ADD
            )
            l1_sb = epip.tile([1, ML], f32)
            l2_sb = epip.tile([1, ML], f32)
            nc.scalar.activation(l1_sb[:], t2_sb[:], AF.Ln)
            nc.scalar.activation(l2_sb[:], S[0:1, :], AF.Ln)
            ld_sb = epip.tile([1, ML], f32)
            nc.vector.tensor_tensor(ld_sb[:], l1_sb[:], l2_sb[:], SUB)
            nc.sync.dma_start(out_ld[:], ld_sb[:])

    nc.compile()
    return nc


def _get_nc():
    if "nc" not in _CACHE:
        _CACHE["nc"] = _build()
    return _CACHE["nc"]


def _split_bf16(a):
    """a (f32) -> (hi, lo) bf16 with hi + lo ~= a to ~2^-16 relative."""
    import ml_dtypes

    hi = a.astype(ml_dtypes.bfloat16)
    lo = (a - hi.astype(np.float32)).astype(ml_dtypes.bfloat16)
    return hi, lo


def _prep_inputs(x, x_ref, y, y_ref):
    import ml_dtypes

    x = np.ascontiguousarray(np.asarray(x, dtype=np.float32))
    x_ref = np.ascontiguousarray(np.asarray(x_ref, dtype=np.float32))
    y = np.asarray(y).astype(np.int64)
    y_ref = np.asarray(y_ref).astype(np.int64)

    # shared across cores ------------------------------------------------
    # reft[b, k, kc*128+n] = hi/lo bf16 of -2 * x_ref[b*128+n, kc*128+k];
    # hi chunks at kc 0..3, lo chunks at kc 4..7
    rhi, rlo = _split_bf16(-2.0 * x_ref)
    parts = []
    for part in (rhi, rlo):
        r4 = part.reshape(NB, 128, KC, 128)               # [b, n, kc, k]
        parts.append(r4.transpose(0, 3, 2, 1).reshape(NB, 128, D))
    reft = np.ascontiguousarray(np.concatenate(parts, axis=2))  # [NB, 128, KK*128]
    r2 = (x_ref.astype(np.float64) ** 2).sum(1).astype(np.float32)  # [N]
    r2t = np.ascontiguousarray(r2.reshape(NB, 128).T)     # [128, NB]
    # ind[p, b*IW+0] = 1 (Z row); ind[p, b*IW+1+c] = 1{y_ref[b*128+p] == c}
    yr = y_ref.reshape(NB, 128)
    indm = np.zeros((NB, 128, IW), dtype=ml_dtypes.bfloat16)
    bi, pi = np.meshgrid(np.arange(NB), np.arange(128), indexing="ij")
    indm[bi, pi, 1 + yr] = 1.0
    indm[:, :, 0] = 1.0
    ind = np.ascontiguousarray(indm.transpose(1, 0, 2).reshape(128, NB * IW))

    in_maps = []
    for c in range(NCORES):
        xc = x[c * ML : (c + 1) * ML]                      # [ML, D]
        # xt[k, kc*ML+m] = xc[m, kc*128+k], hi chunks then lo chunks
        xhi, xlo = _split_bf16(xc)
        xt = np.ascontiguousarray(
            np.concatenate(
                [
                    p.reshape(ML, KC, 128).transpose(2, 1, 0).reshape(128, KC * ML)
                    for p in (xhi, xlo)
                ],
                axis=1,
            )
        )
        x2 = (xc.astype(np.float64) ** 2).sum(1).astype(np.float32)  # [ML]
        x2b = np.ascontiguousarray(np.broadcast_to(x2, (128, ML)))
        yc = y[c * ML : (c + 1) * ML]
        ybm = np.zeros((IW, ML), dtype=np.float32)
        ybm[1 + yc, np.arange(ML)] = 1.0
        in_maps.append(
            {
                "reft": reft,
                "xt": xt,
                "x2b": x2b,
                "r2t": r2t,
                "ind": ind,
                "yb": ybm,
            }
        )
    return in_maps


def run(x, x_ref, y, y_ref, trace=False, trace_kwargs=None):
    from concourse.bass_utils import run_bass_kernel_spmd

    nc = _get_nc()
    in_maps = _prep_inputs(x, x_ref, y, y_ref)
    res = run_bass_kernel_spmd(
        nc,
        in_maps,
        list(range(NCORES)),
        trace=trace,
        **(trace_kwargs or {}),
    )
    ld = np.concatenate([res.results[c]["out_ld"].reshape(-1) for c in range(NCORES)])
    loss = np.float32(-(ld.astype(np.float64).mean()))
    return loss, res


def kernel(x, x_ref, y, y_ref):
    loss, _ = run(x, x_ref, y, y_ref)
    return np.asarray(loss, dtype=np.float32)


# revision 20
# speedup vs baseline: 2.9387x; 1.3937x over previous
"""ASK loss (soft nearest-neighbor NLL) on 8 Trainium2 NeuronCores.

Math (matches the jax reference exactly):
    dist[m,n]  = sqrt(||x_m||^2 + ||r_n||^2 - 2 x_m.r_n)
    score      = softmax(-dist, axis=n)
    soft_nns   = segment_sum(score over classes of y_ref) + EPS
    loss       = -mean_m log(soft_nns[m, y[m]])

Because the softmax normalizer cancels in the ratio, we compute with
unnormalized E = exp(-dist):
    S[c,m] = sum_{n: y_ref[n]=c} E[m,n],   Z[m] = sum_n E[m,n]
    loss_m = log(S[y_m,m] + EPS*Z[m]) - log(Z[m]);  loss = -mean loss_m

Sharding: data-parallel over the batch M. Each core owns 512 rows of x and
streams the full (pre-transposed, -2-scaled) reference set.

Per-core device pipeline, per 128-reference block b (256 blocks):
    PE : psum[n,m] = sum_kc (-2 ref^T chunk).T @ x^T chunk   (4 float32r
         matmuls: fp32-container reduced-product mode that streams at bf16
         rate, ~12-bit effective mantissa -- ample for the v error budget)
    DVE: v = (psum + r2[n]) + x2bcast[m]                     (1 fused STT op)
    ACT: v -> ln -> exp(0.5.) -> exp(-1.) == exp(-sqrt(v))   (wide, in-place,
         single act table set: natural_log_exp_and_others; last exp emits
         bf16 for the reduction matmul)
    PE : S[0:11, m] += onehot(y_ref)^T @ E                   (bf16, psum acc)
Epilogue picks S[y_m, m] via a one-hot of y, takes logs on ACT, and DMAs
per-row log-probs out; the host averages 8x512 values.
"""

import numpy as np

M, N, D = 4096, 32768, 512
NCORES = 8
ML = M // NCORES          # 512 rows of x per core
NB = N // 128             # 256 reference blocks
KC = D // 128             # 4 contraction chunks
NCLS = 10
EPS = 1e-6
W = 8                     # blocks fused into one wide ACT group
IW = NCLS + 1             # indicator cols per block: col 0 = ones (Z), 1+c = class c
KK = 2 * KC               # bf16 hi/lo split doubles the contraction chunks

_CACHE = {}


def _patch_act_tables():
    """Restrict Ln/Exp membership to natural_log_exp_and_others so bacc's
    greedy table chooser emits exactly one ACT table load for the kernel.
    Entry order (= act_func_set_id) is preserved; the actually-loaded set
    genuinely contains both functions."""
    import concourse.bacc as bacc_mod
    import concourse.hw_specs as hw_specs
    import concourse.mybir as mybir

    real = hw_specs.get_activation_tables

    def patched(arch):
        tabs = dict(real(arch))
        ln = mybir.ActivationFunctionType.Ln
        ex = mybir.ActivationFunctionType.Exp
        out = {}
        for name, fns in tabs.items():
            if name != "natural_log_exp_and_others":
                fns = fns - {ln, ex}
            out[name] = fns
        return out

    bacc_mod.get_activation_tables = patched


def _build():
    import concourse.bass as bass
    import concourse.bacc as bacc
    import concourse.mybir as mybir
    import concourse.tile as tile

    _patch_act_tables()

    f32 = mybir.dt.float32
    f32r = mybir.dt.float32r
    bf16 = mybir.dt.bfloat16
    AF = mybir.ActivationFunctionType
    ADD = mybir.AluOpType.add
    MUL = mybir.AluOpType.mult
    SUB = mybir.AluOpType.subtract

    nc = bacc.Bacc("TRN2", target_bir_lowering=False, debug=False)

    reft = nc.dram_tensor("reft", [NB, 128, D], f32r, kind="ExternalInput").ap()
    xt = nc.dram_tensor("xt", [128, KC * ML], f32r, kind="ExternalInput").ap()
    x2b = nc.dram_tensor("x2b", [128, ML], f32, kind="ExternalInput").ap()
    r2t = nc.dram_tensor("r2t", [128, NB], f32, kind="ExternalInput").ap()
    ind = nc.dram_tensor("ind", [128, NB * IW], bf16, kind="ExternalInput").ap()
    yb = nc.dram_tensor("yb", [IW, ML], f32, kind="ExternalInput").ap()
    out_ld = nc.dram_tensor("out_ld", [1, ML], f32, kind="ExternalOutput").ap()

    with tile.TileContext(nc) as tc:
        with (
            tc.tile_pool(name="const", bufs=1) as constp,
            tc.tile_pool(name="refp", bufs=4) as refp,
            tc.tile_pool(name="wide", bufs=3) as widep,
            tc.tile_pool(name="epi", bufs=1) as epip,
            tc.tile_pool(name="pdot", bufs=4, space=bass.MemorySpace.PSUM) as pdot,
            tc.tile_pool(name="pacc", bufs=1, space=bass.MemorySpace.PSUM) as pacc,
            tc.tile_pool(name="pone", bufs=1, space=bass.MemorySpace.PSUM) as pone,
        ):
            xt_sb = constp.tile([128, KC * ML], f32r)
            x2b_sb = constp.tile([128, ML], f32)
            r2_sb = constp.tile([128, NB], f32)
            ind_sb = constp.tile([128, NB * IW], bf16)
            yb_sb = constp.tile([IW, ML], f32)
            ones_sb = constp.tile([IW, 1], f32)
            nc.sync.dma_start(xt_sb[:], xt[:])
            nc.sync.dma_start(x2b_sb[:], x2b[:])
            nc.sync.dma_start(r2_sb[:], r2t[:])
            nc.sync.dma_start(ind_sb[:], ind[:])
            nc.sync.dma_start(yb_sb[:], yb[:])
            nc.vector.memset(ones_sb[:], 1.0)

            S = pacc.tile([IW, ML], f32)

            for g in range(NB // W):
                s_w = widep.tile([128, W * ML], f32)
                e_w = widep.tile([128, W * ML], bf16, tag="e_w")
                for w in range(W):
                    b = g * W + w
                    ref_sb = refp.tile([128, D], f32r)
                    nc.sync.dma_start(ref_sb[:], reft[b])
                    pd = pdot.tile([128, ML], f32)
                    for kc in range(KC):
                        nc.tensor.matmul(
                            pd[:],
                            ref_sb[:, kc * 128 : (kc + 1) * 128],
                            xt_sb[:, kc * ML : (kc + 1) * ML],
                            start=(kc == 0),
                            stop=(kc == KC - 1),
                        )
                    # v = (dot + r2[n]) + x2[m]  (r2 per-partition, x2 bcast)
                    nc.vector.scalar_tensor_tensor(
                        s_w[:, w * ML : (w + 1) * ML],
                        pd[:],
                        r2_sb[:, b : b + 1],
                        x2b_sb[:],
                        op0=ADD,
                        op1=ADD,
                    )
                # E = exp(-sqrt(v)) = exp(-exp(0.5*ln(v))), wide; last exp
                # downcasts to bf16 for the reduction matmul
                nc.scalar.activation(s_w[:], s_w[:], AF.Ln)
                nc.scalar.activation(s_w[:], s_w[:], AF.Exp, scale=0.5)
                nc.scalar.activation(e_w[:], s_w[:], AF.Exp, scale=-1.0)
                for w in range(W):
                    b = g * W + w
                    nc.tensor.matmul(
                        S[:],
                        ind_sb[:, b * IW : (b + 1) * IW],
                        e_w[:, w * ML : (w + 1) * ML],
                        start=(b == 0),
                        stop=(b == NB - 1),
                    )

            # ---- epilogue: loss_m = log(S[y_m] + EPS*Z) - log(Z) ----
            # S row 0 = Z (ones column of the indicator); rows 1..10 = classes.
            # yb row 0 is zero, so the full-11-partition ops never slice at
            # partition offsets the BIR verifier rejects.
            t_sb = epip.tile([IW, ML], f32)
            nc.vector.tensor_tensor(t_sb[:], S[:], yb_sb[:], MUL)
            p0 = pone.tile([1, ML], f32)
            nc.tensor.matmul(p0[:], ones_sb[:], t_sb[:], start=True, stop=True)
            p0_sb = epip.tile([1, ML], f32)
            nc.scalar.copy(p0_sb[:], p0[:])
            t2_sb = epip.tile([1, ML], f32)
            # t2 = (Z * EPS) + p0
            nc.vector.scalar_tensor_tensor(
                t2_sb[:], S[0:1, :], float(EPS), p0_sb[:], op0=MUL, op1=ADD
            )
            l1_sb = epip.tile([1, ML], f32)
            l2_sb = epip.tile([1, ML], f32)
            nc.scalar.activation(l1_sb[:], t2_sb[:], AF.Ln)
            nc.scalar.activation(l2_sb[:], S[0:1, :], AF.Ln)
            ld_sb = epip.tile([1, ML], f32)
            nc.vector.tensor_tensor(ld_sb[:], l1_sb[:], l2_sb[:], SUB)
            nc.sync.dma_start(out_ld[:], ld_sb[:])

    nc.compile()
    return nc


def _get_nc():
    if "nc" not in _CACHE:
        _CACHE["nc"] = _build()
    return _CACHE["nc"]


def _prep_inputs(x, x_ref, y, y_ref):
    import ml_dtypes

    x = np.ascontiguousarray(np.asarray(x, dtype=np.float32))
    x_ref = np.ascontiguousarray(np.asarray(x_ref, dtype=np.float32))
    y = np.asarray(y).astype(np.int64)
    y_ref = np.asarray(y_ref).astype(np.int64)

    # shared across cores ------------------------------------------------
    # reft[b, k, kc*128+n] = -2 * x_ref[b*128+n, kc*128+k]  (fed as float32r)
    r4 = (-2.0 * x_ref).reshape(NB, 128, KC, 128)         # [b, n, kc, k]
    reft = np.ascontiguousarray(r4.transpose(0, 3, 2, 1).reshape(NB, 128, D))
    r2 = (x_ref.astype(np.float64) ** 2).sum(1).astype(np.float32)  # [N]
    r2t = np.ascontiguousarray(r2.reshape(NB, 128).T)     # [128, NB]
    # ind[p, b*IW+0] = 1 (Z row); ind[p, b*IW+1+c] = 1{y_ref[b*128+p] == c}
    yr = y_ref.reshape(NB, 128)
    indm = np.zeros((NB, 128, IW), dtype=ml_dtypes.bfloat16)
    bi, pi = np.meshgrid(np.arange(NB), np.arange(128), indexing="ij")
    indm[bi, pi, 1 + yr] = 1.0
    indm[:, :, 0] = 1.0
    ind = np.ascontiguousarray(indm.transpose(1, 0, 2).reshape(128, NB * IW))

    in_maps = []
    for c in range(NCORES):
        xc = x[c * ML : (c + 1) * ML]                      # [ML, D]
        # xt[k, kc*ML+m] = xc[m, kc*128+k]  (fed as float32r)
        xt = np.ascontiguousarray(
            xc.reshape(ML, KC, 128).transpose(2, 1, 0).reshape(128, KC * ML)
        )
        x2 = (xc.astype(np.float64) ** 2).sum(1).astype(np.float32)  # [ML]
        x2b = np.ascontiguousarray(np.broadcast_to(x2, (128, ML)))
        yc = y[c * ML : (c + 1) * ML]
        ybm = np.zeros((IW, ML), dtype=np.float32)
        ybm[1 + yc, np.arange(ML)] = 1.0
        in_maps.append(
            {
                "reft": reft,
                "xt": xt,
                "x2b": x2b,
                "r2t": r2t,
                "ind": ind,
                "yb": ybm,
            }
        )
    return in_maps


def run(x, x_ref, y, y_ref, trace=False, trace_kwargs=None):
    from concourse.bass_utils import run_bass_kernel_spmd

    nc = _get_nc()
    in_maps = _prep_inputs(x, x_ref, y, y_ref)
    res = run_bass_kernel_spmd(
        nc,
        in_maps,
        list(range(NCORES)),
        trace=trace,
        **(trace_kwargs or {}),
    )
    ld = np.concatenate([res.results[c]["out_ld"].reshape(-1) for c in range(NCORES)])
    loss = np.float32(-(ld.astype(np.float64).mean()))
    return loss, res


def kernel(x, x_ref, y, y_ref):
    loss, _ = run(x, x_ref, y, y_ref)
    return np.asarray(loss, dtype=np.float32)


# revision 31
# speedup vs baseline: 3.2088x; 1.0919x over previous
"""ASK loss (soft nearest-neighbor NLL) on 8 Trainium2 NeuronCores.

Math (matches the jax reference exactly):
    dist[m,n]  = sqrt(||x_m||^2 + ||r_n||^2 - 2 x_m.r_n)
    score      = softmax(-dist, axis=n)
    soft_nns   = segment_sum(score over classes of y_ref) + EPS
    loss       = -mean_m log(soft_nns[m, y[m]])

Because the softmax normalizer cancels in the ratio, we compute with
unnormalized E = exp(-dist):
    S[c,m] = sum_{n: y_ref[n]=c} E[m,n],   Z[m] = sum_n E[m,n]
    loss_m = log(S[y_m,m] + EPS*Z[m]) - log(Z[m]);  loss = -mean loss_m

Sharding: data-parallel over the batch M. Each core owns 512 rows of x and
streams the full (pre-transposed, -2-scaled) reference set.

Per-core device pipeline, per 128-reference block b (256 blocks):
    PE : psum[n,m] = sum_kc (-2 ref^T chunk).T @ x^T chunk   (4 float32r
         matmuls: fp32-container reduced-product mode that streams at bf16
         rate, ~12-bit effective mantissa -- ample for the v error budget)
    DVE: v = (psum + r2[n]) + x2bcast[m]                     (1 fused STT op)
    ACT: d = sqrt(v) in place, then E = exp(-d) -> bf16.  Sqrt and Exp live
         in different ACT table sets (~2.7us per switch), so the schedule
         batches B groups of sqrts then B groups of exps: 16 switches total.
    PE : S[0:11, m] += onehot(y_ref)^T @ E                   (bf16, psum acc)
Epilogue picks S[y_m, m] via a one-hot of y, takes logs on ACT, and DMAs
per-row log-probs out; the host averages 8x512 values.
"""

import numpy as np

M, N, D = 4096, 32768, 512
NCORES = 8
ML = M // NCORES          # 512 rows of x per core
NB = N // 128             # 256 reference blocks
KC = D // 128             # 4 contraction chunks
NCLS = 10
EPS = 1e-6
W = 4                     # blocks fused into one wide ACT group
B = 8                     # ACT groups per table-set batch (sqrt x B, then exp x B)
IW = NCLS + 1             # indicator cols per block: col 0 = ones (Z), 1+c = class c

_CACHE = {}


def _patch_act_tables():
    """Restrict Ln/Exp membership to natural_log_exp_and_others so bacc's
    greedy table chooser emits exactly one ACT table load for the kernel.
    Entry order (= act_func_set_id) is preserved; the actually-loaded set
    genuinely contains both functions."""
    import concourse.bacc as bacc_mod
    import concourse.hw_specs as hw_specs
    import concourse.mybir as mybir

    real = hw_specs.get_activation_tables

    def patched(arch):
        tabs = dict(real(arch))
        ln = mybir.ActivationFunctionType.Ln
        ex = mybir.ActivationFunctionType.Exp
        sq = mybir.ActivationFunctionType.Sqrt
        out = {}
        for name, fns in tabs.items():
            if name != "natural_log_exp_and_others":
                fns = fns - {ln, ex}
            if name != "sqrt_and_others":
                fns = fns - {sq}
            out[name] = fns
        return out

    bacc_mod.get_activation_tables = patched


def _build():
    import concourse.bass as bass
    import concourse.bacc as bacc
    import concourse.mybir as mybir
    import concourse.tile as tile
    from concourse.tile import add_dep_helper

    _patch_act_tables()

    f32 = mybir.dt.float32
    f32r = mybir.dt.float32r
    bf16 = mybir.dt.bfloat16
    AF = mybir.ActivationFunctionType
    ADD = mybir.AluOpType.add
    MUL = mybir.AluOpType.mult
    SUB = mybir.AluOpType.subtract

    nc = bacc.Bacc("TRN2", target_bir_lowering=False, debug=False)

    reft = nc.dram_tensor("reft", [NB, 128, D], f32r, kind="ExternalInput").ap()
    xt = nc.dram_tensor("xt", [128, KC * ML], f32r, kind="ExternalInput").ap()
    x2b = nc.dram_tensor("x2b", [128, ML], f32, kind="ExternalInput").ap()
    r2t = nc.dram_tensor("r2t", [128, NB], f32, kind="ExternalInput").ap()
    ind = nc.dram_tensor("ind", [128, NB * IW], bf16, kind="ExternalInput").ap()
    yb = nc.dram_tensor("yb", [IW, ML], f32, kind="ExternalInput").ap()
    out_ld = nc.dram_tensor("out_ld", [1, ML], f32, kind="ExternalOutput").ap()

    with tile.TileContext(nc) as tc:
        with (
            tc.tile_pool(name="const", bufs=1) as constp,
            tc.tile_pool(name="refp", bufs=6) as refp,
            tc.tile_pool(name="wide", bufs=2 * B + 1) as widep,
            tc.tile_pool(name="ew", bufs=4) as ewp,
            tc.tile_pool(name="epi", bufs=1) as epip,
            tc.tile_pool(name="pdot", bufs=4, space=bass.MemorySpace.PSUM) as pdot,
            tc.tile_pool(name="pacc", bufs=1, space=bass.MemorySpace.PSUM) as pacc,
            tc.tile_pool(name="pone", bufs=1, space=bass.MemorySpace.PSUM) as pone,
        ):
            xt_sb = constp.tile([128, KC * ML], f32r)
            x2b_sb = constp.tile([128, ML], f32)
            r2_sb = constp.tile([128, NB], f32)
            ind_sb = constp.tile([128, NB * IW], bf16)
            yb_sb = constp.tile([IW, ML], f32)
            ones_sb = constp.tile([IW, 1], f32)
            nc.sync.dma_start(xt_sb[:], xt[:])
            nc.sync.dma_start(x2b_sb[:], x2b[:])
            nc.sync.dma_start(r2_sb[:], r2t[:])
            nc.sync.dma_start(ind_sb[:], ind[:])
            nc.sync.dma_start(yb_sb[:], yb[:])
            nc.vector.memset(ones_sb[:], 1.0)

            S = pacc.tile([IW, ML], f32)

            n_groups = NB // W
            # Totally order ACT instructions in program order so the table
            # loads stay batched (scheduler must not interleave sqrt/exp
            # phases). Ordering a serial engine costs nothing.
            act_chain = [None]

            def act(*args, **kwargs):
                inst = nc.scalar.activation(*args, **kwargs)
                if act_chain[0] is not None:
                    add_dep_helper(
                        inst.ins, act_chain[0].ins, sync=False,
                        reason="ACT program order (table-set batching)",
                    )
                act_chain[0] = inst
                return inst

            for batch in range(n_groups // B):
                batch_sw = []
                for gg in range(B):
                    g = batch * B + gg
                    s_w = widep.tile([128, W * ML], f32)
                    for w in range(W):
                        b = g * W + w
                        ref_sb = refp.tile([128, D], f32r)
                        nc.sync.dma_start(ref_sb[:], reft[b])
                        pd = pdot.tile([128, ML], f32)
                        for kc in range(KC):
                            nc.tensor.matmul(
                                pd[:],
                                ref_sb[:, kc * 128 : (kc + 1) * 128],
                                xt_sb[:, kc * ML : (kc + 1) * ML],
                                start=(kc == 0),
                                stop=(kc == KC - 1),
                            )
                        # v = (dot + r2[n]) + x2[m]
                        nc.vector.scalar_tensor_tensor(
                            s_w[:, w * ML : (w + 1) * ML],
                            pd[:],
                            r2_sb[:, b : b + 1],
                            x2b_sb[:],
                            op0=ADD,
                            op1=ADD,
                        )
                    # d = sqrt(v), in place (sqrt table set, batched)
                    act(s_w[:], s_w[:], AF.Sqrt)
                    batch_sw.append((g, s_w))
                for g, s_w in batch_sw:
                    # E = exp(-d), downcast to bf16 (exp table set, batched)
                    e_w = ewp.tile([128, W * ML], bf16)
                    act(e_w[:], s_w[:], AF.Exp, scale=-1.0)
                    for w in range(W):
                        b = g * W + w
                        nc.tensor.matmul(
                            S[:],
                            ind_sb[:, b * IW : (b + 1) * IW],
                            e_w[:, w * ML : (w + 1) * ML],
                            start=(b == 0),
                            stop=(b == NB - 1),
                        )

            # ---- epilogue: loss_m = log(S[y_m] + EPS*Z) - log(Z) ----
            # S row 0 = Z (ones column of the indicator); rows 1..10 = classes.
            # yb row 0 is zero, so the full-11-partition ops never slice at
            # partition offsets the BIR verifier rejects.
            t_sb = epip.tile([IW, ML], f32)
            nc.vector.tensor_tensor(t_sb[:], S[:], yb_sb[:], MUL)
            p0 = pone.tile([1, ML], f32)
            nc.tensor.matmul(p0[:], ones_sb[:], t_sb[:], start=True, stop=True)
            p0_sb = epip.tile([1, ML], f32)
            act(p0_sb[:], p0[:], AF.Copy)
            t2_sb = epip.tile([1, ML], f32)
            # t2 = (Z * EPS) + p0
            nc.vector.scalar_tensor_tensor(
                t2_sb[:], S[0:1, :], float(EPS), p0_sb[:], op0=MUL, op1=ADD
            )
            l1_sb = epip.tile([1, ML], f32)
            l2_sb = epip.tile([1, ML], f32)
            act(l1_sb[:], t2_sb[:], AF.Ln)
            act(l2_sb[:], S[0:1, :], AF.Ln)
            ld_sb = epip.tile([1, ML], f32)
            nc.vector.tensor_tensor(ld_sb[:], l1_sb[:], l2_sb[:], SUB)
            nc.sync.dma_start(out_ld[:], ld_sb[:])

    nc.compile()
    return nc


def _get_nc():
    if "nc" not in _CACHE:
        _CACHE["nc"] = _build()
    return _CACHE["nc"]


def _prep_inputs(x, x_ref, y, y_ref):
    import ml_dtypes

    x = np.ascontiguousarray(np.asarray(x, dtype=np.float32))
    x_ref = np.ascontiguousarray(np.asarray(x_ref, dtype=np.float32))
    y = np.asarray(y).astype(np.int64)
    y_ref = np.asarray(y_ref).astype(np.int64)

    # shared across cores ------------------------------------------------
    # reft[b, k, kc*128+n] = -2 * x_ref[b*128+n, kc*128+k]  (fed as float32r)
    r4 = (-2.0 * x_ref).reshape(NB, 128, KC, 128)         # [b, n, kc, k]
    reft = np.ascontiguousarray(r4.transpose(0, 3, 2, 1).reshape(NB, 128, D))
    r2 = (x_ref.astype(np.float64) ** 2).sum(1).astype(np.float32)  # [N]
    r2t = np.ascontiguousarray(r2.reshape(NB, 128).T)     # [128, NB]
    # ind[p, b*IW+0] = 1 (Z row); ind[p, b*IW+1+c] = 1{y_ref[b*128+p] == c}
    yr = y_ref.reshape(NB, 128)
    indm = np.zeros((NB, 128, IW), dtype=ml_dtypes.bfloat16)
    bi, pi = np.meshgrid(np.arange(NB), np.arange(128), indexing="ij")
    indm[bi, pi, 1 + yr] = 1.0
    indm[:, :, 0] = 1.0
    ind = np.ascontiguousarray(indm.transpose(1, 0, 2).reshape(128, NB * IW))

    in_maps = []
    for c in range(NCORES):
        xc = x[c * ML : (c + 1) * ML]                      # [ML, D]
        # xt[k, kc*ML+m] = xc[m, kc*128+k]  (fed as float32r)
        xt = np.ascontiguousarray(
            xc.reshape(ML, KC, 128).transpose(2, 1, 0).reshape(128, KC * ML)
        )
        x2 = (xc.astype(np.float64) ** 2).sum(1).astype(np.float32)  # [ML]
        x2b = np.ascontiguousarray(np.broadcast_to(x2, (128, ML)))
        yc = y[c * ML : (c + 1) * ML]
        ybm = np.zeros((IW, ML), dtype=np.float32)
        ybm[1 + yc, np.arange(ML)] = 1.0
        in_maps.append(
            {
                "reft": reft,
                "xt": xt,
                "x2b": x2b,
                "r2t": r2t,
                "ind": ind,
                "yb": ybm,
            }
        )
    return in_maps


def run(x, x_ref, y, y_ref, trace=False, trace_kwargs=None):
    from concourse.bass_utils import run_bass_kernel_spmd

    nc = _get_nc()
    in_maps = _prep_inputs(x, x_ref, y, y_ref)
    res = run_bass_kernel_spmd(
        nc,
        in_maps,
        list(range(NCORES)),
        trace=trace,
        **(trace_kwargs or {}),
    )
    ld = np.concatenate([res.results[c]["out_ld"].reshape(-1) for c in range(NCORES)])
    loss = np.float32(-(ld.astype(np.float64).mean()))
    return loss, res


def kernel(x, x_ref, y, y_ref):
    loss, _ = run(x, x_ref, y, y_ref)
    return np.asarray(loss, dtype=np.float32)


# revision 33
# speedup vs baseline: 3.2760x; 1.0209x over previous
"""ASK loss (soft nearest-neighbor NLL) on 8 Trainium2 NeuronCores.

Math (matches the jax reference exactly):
    dist[m,n]  = sqrt(||x_m||^2 + ||r_n||^2 - 2 x_m.r_n)
    score      = softmax(-dist, axis=n)
    soft_nns   = segment_sum(score over classes of y_ref) + EPS
    loss       = -mean_m log(soft_nns[m, y[m]])

Because the softmax normalizer cancels in the ratio, we compute with
unnormalized E = exp(-dist):
    S[c,m] = sum_{n: y_ref[n]=c} E[m,n],   Z[m] = sum_n E[m,n]
    loss_m = log(S[y_m,m] + EPS*Z[m]) - log(Z[m]);  loss = -mean loss_m

Sharding: data-parallel over the batch M. Each core owns 512 rows of x and
streams the full (pre-transposed, -2-scaled) reference set.

Per-core device pipeline, per 128-reference block b (256 blocks):
    PE : psum[n,m] = sum_kc (-2 ref^T chunk).T @ x^T chunk   (4 float32r
         matmuls: fp32-container reduced-product mode that streams at bf16
         rate, ~12-bit effective mantissa -- ample for the v error budget)
    DVE: v = (psum + r2[n]) + x2bcast[m]                     (1 fused STT op)
    ACT: d = sqrt(v) in place, then E = exp(-d) -> bf16.  Sqrt and Exp live
         in different ACT table sets (~2.7us per switch), so the schedule
         batches B groups of sqrts then B groups of exps: 16 switches total.
    PE : S[0:11, m] += onehot(y_ref)^T @ E                   (bf16, psum acc)
Epilogue picks S[y_m, m] via a one-hot of y, takes logs on ACT, and DMAs
per-row log-probs out; the host averages 8x512 values.
"""

import numpy as np

M, N, D = 4096, 32768, 512
NCORES = 8
ML = M // NCORES          # 512 rows of x per core
NB = N // 128             # 256 reference blocks
KC = D // 128             # 4 contraction chunks
NCLS = 10
EPS = 1e-6
W = 4                     # blocks fused into one wide ACT group
B = 8                     # ACT groups per table-set batch (sqrt x B, then exp x B)
IW = NCLS + 1             # indicator cols per block: col 0 = ones (Z), 1+c = class c

_CACHE = {}


def _patch_act_tables():
    """Restrict Ln/Exp membership to natural_log_exp_and_others so bacc's
    greedy table chooser emits exactly one ACT table load for the kernel.
    Entry order (= act_func_set_id) is preserved; the actually-loaded set
    genuinely contains both functions."""
    import concourse.bacc as bacc_mod
    import concourse.hw_specs as hw_specs
    import concourse.mybir as mybir

    real = hw_specs.get_activation_tables

    def patched(arch):
        tabs = dict(real(arch))
        ln = mybir.ActivationFunctionType.Ln
        ex = mybir.ActivationFunctionType.Exp
        sq = mybir.ActivationFunctionType.Sqrt
        out = {}
        for name, fns in tabs.items():
            if name != "natural_log_exp_and_others":
                fns = fns - {ln, ex}
            if name != "sqrt_and_others":
                fns = fns - {sq}
            out[name] = fns
        return out

    bacc_mod.get_activation_tables = patched


def _build():
    import concourse.bass as bass
    import concourse.bacc as bacc
    import concourse.mybir as mybir
    import concourse.tile as tile
    from concourse.tile import add_dep_helper

    _patch_act_tables()

    f32 = mybir.dt.float32
    f32r = mybir.dt.float32r
    bf16 = mybir.dt.bfloat16
    AF = mybir.ActivationFunctionType
    ADD = mybir.AluOpType.add
    MUL = mybir.AluOpType.mult
    SUB = mybir.AluOpType.subtract

    nc = bacc.Bacc("TRN2", target_bir_lowering=False, debug=False)

    reft = nc.dram_tensor("reft", [NB, 128, D], f32r, kind="ExternalInput").ap()
    xt = nc.dram_tensor("xt", [128, KC * ML], f32r, kind="ExternalInput").ap()
    x2b = nc.dram_tensor("x2b", [128, ML], f32, kind="ExternalInput").ap()
    r2t = nc.dram_tensor("r2t", [128, NB], f32, kind="ExternalInput").ap()
    ind = nc.dram_tensor("ind", [128, NB * IW], bf16, kind="ExternalInput").ap()
    yb = nc.dram_tensor("yb", [IW, ML], f32, kind="ExternalInput").ap()
    out_ld = nc.dram_tensor("out_ld", [1, ML], f32, kind="ExternalOutput").ap()

    with tile.TileContext(nc) as tc:
        with (
            tc.tile_pool(name="const", bufs=1) as constp,
            tc.tile_pool(name="refp", bufs=8) as refp,
            tc.tile_pool(name="wide", bufs=2 * B) as widep,
            tc.tile_pool(name="ew", bufs=5) as ewp,
            tc.tile_pool(name="epi", bufs=1) as epip,
            tc.tile_pool(name="pdot", bufs=6, space=bass.MemorySpace.PSUM) as pdot,
            tc.tile_pool(name="pacc", bufs=1, space=bass.MemorySpace.PSUM) as pacc,
            tc.tile_pool(name="pone", bufs=1, space=bass.MemorySpace.PSUM) as pone,
        ):
            xt_sb = constp.tile([128, KC * ML], f32r)
            x2b_sb = constp.tile([128, ML], f32)
            r2_sb = constp.tile([128, NB], f32)
            ind_sb = constp.tile([128, NB * IW], bf16)
            yb_sb = constp.tile([IW, ML], f32)
            ones_sb = constp.tile([IW, 1], f32)
            # chunked so the first matmuls are not gated on the whole tensor;
            # late consts (ind/yb) queue behind the first ref-block loads
            for kc in range(KC):
                nc.sync.dma_start(
                    xt_sb[:, kc * ML : (kc + 1) * ML], xt[:, kc * ML : (kc + 1) * ML]
                )
            nc.sync.dma_start(x2b_sb[:], x2b[:])
            nc.sync.dma_start(r2_sb[:], r2t[:])
            nc.vector.memset(ones_sb[:], 1.0)

            S = pacc.tile([IW, ML], f32)

            n_groups = NB // W
            # Totally order ACT instructions in program order so the table
            # loads stay batched (scheduler must not interleave sqrt/exp
            # phases). Ordering a serial engine costs nothing.
            act_chain = [None]

            def act(*args, **kwargs):
                inst = nc.scalar.activation(*args, **kwargs)
                if act_chain[0] is not None:
                    add_dep_helper(
                        inst.ins, act_chain[0].ins, sync=False,
                        reason="ACT program order (table-set batching)",
                    )
                act_chain[0] = inst
                return inst

            for batch in range(n_groups // B):
                batch_sw = []
                for gg in range(B):
                    g = batch * B + gg
                    if batch == 0 and gg == 1:
                        # late-needed constants, behind the first ref loads
                        nc.sync.dma_start(ind_sb[:], ind[:])
                        nc.sync.dma_start(yb_sb[:], yb[:])
                    s_w = widep.tile([128, W * ML], f32)
                    for w in range(W):
                        b = g * W + w
                        ref_sb = refp.tile([128, D], f32r)
                        nc.sync.dma_start(ref_sb[:], reft[b])
                        pd = pdot.tile([128, ML], f32)
                        for kc in range(KC):
                            nc.tensor.matmul(
                                pd[:],
                                ref_sb[:, kc * 128 : (kc + 1) * 128],
                                xt_sb[:, kc * ML : (kc + 1) * ML],
                                start=(kc == 0),
                                stop=(kc == KC - 1),
                            )
                        # v = (dot + r2[n]) + x2[m]
                        nc.vector.scalar_tensor_tensor(
                            s_w[:, w * ML : (w + 1) * ML],
                            pd[:],
                            r2_sb[:, b : b + 1],
                            x2b_sb[:],
                            op0=ADD,
                            op1=ADD,
                        )
                    # d = sqrt(v), in place (sqrt table set, batched)
                    act(s_w[:], s_w[:], AF.Sqrt)
                    batch_sw.append((g, s_w))
                for g, s_w in batch_sw:
                    # E = exp(-d), downcast to bf16 (exp table set, batched)
                    e_w = ewp.tile([128, W * ML], bf16)
                    act(e_w[:], s_w[:], AF.Exp, scale=-1.0)
                    for w in range(W):
                        b = g * W + w
                        nc.tensor.matmul(
                            S[:],
                            ind_sb[:, b * IW : (b + 1) * IW],
                            e_w[:, w * ML : (w + 1) * ML],
                            start=(b == 0),
                            stop=(b == NB - 1),
                        )

            # ---- epilogue: loss_m = log(S[y_m] + EPS*Z) - log(Z) ----
            # S row 0 = Z (ones column of the indicator); rows 1..10 = classes.
            # yb row 0 is zero, so the full-11-partition ops never slice at
            # partition offsets the BIR verifier rejects.
            t_sb = epip.tile([IW, ML], f32)
            nc.vector.tensor_tensor(t_sb[:], S[:], yb_sb[:], MUL)
            p0 = pone.tile([1, ML], f32)
            nc.tensor.matmul(p0[:], ones_sb[:], t_sb[:], start=True, stop=True)
            p0_sb = epip.tile([1, ML], f32)
            act(p0_sb[:], p0[:], AF.Copy)
            t2_sb = epip.tile([1, ML], f32)
            # t2 = (Z * EPS) + p0
            nc.vector.scalar_tensor_tensor(
                t2_sb[:], S[0:1, :], float(EPS), p0_sb[:], op0=MUL, op1=ADD
            )
            l1_sb = epip.tile([1, ML], f32)
            l2_sb = epip.tile([1, ML], f32)
            act(l1_sb[:], t2_sb[:], AF.Ln)
            act(l2_sb[:], S[0:1, :], AF.Ln)
            ld_sb = epip.tile([1, ML], f32)
            nc.vector.tensor_tensor(ld_sb[:], l1_sb[:], l2_sb[:], SUB)
            nc.sync.dma_start(out_ld[:], ld_sb[:])

    nc.compile()
    return nc


def _get_nc():
    if "nc" not in _CACHE:
        _CACHE["nc"] = _build()
    return _CACHE["nc"]


def _prep_inputs(x, x_ref, y, y_ref):
    import ml_dtypes

    x = np.ascontiguousarray(np.asarray(x, dtype=np.float32))
    x_ref = np.ascontiguousarray(np.asarray(x_ref, dtype=np.float32))
    y = np.asarray(y).astype(np.int64)
    y_ref = np.asarray(y_ref).astype(np.int64)

    # shared across cores ------------------------------------------------
    # reft[b, k, kc*128+n] = -2 * x_ref[b*128+n, kc*128+k]  (fed as float32r)
    r4 = (-2.0 * x_ref).reshape(NB, 128, KC, 128)         # [b, n, kc, k]
    reft = np.ascontiguousarray(r4.transpose(0, 3, 2, 1).reshape(NB, 128, D))
    r2 = (x_ref.astype(np.float64) ** 2).sum(1).astype(np.float32)  # [N]
    r2t = np.ascontiguousarray(r2.reshape(NB, 128).T)     # [128, NB]
    # ind[p, b*IW+0] = 1 (Z row); ind[p, b*IW+1+c] = 1{y_ref[b*128+p] == c}
    yr = y_ref.reshape(NB, 128)
    indm = np.zeros((NB, 128, IW), dtype=ml_dtypes.bfloat16)
    bi, pi = np.meshgrid(np.arange(NB), np.arange(128), indexing="ij")
    indm[bi, pi, 1 + yr] = 1.0
    indm[:, :, 0] = 1.0
    ind = np.ascontiguousarray(indm.transpose(1, 0, 2).reshape(128, NB * IW))

    in_maps = []
    for c in range(NCORES):
        xc = x[c * ML : (c + 1) * ML]                      # [ML, D]
        # xt[k, kc*ML+m] = xc[m, kc*128+k]  (fed as float32r)
        xt = np.ascontiguousarray(
            xc.reshape(ML, KC, 128).transpose(2, 1, 0).reshape(128, KC * ML)
        )
        x2 = (xc.astype(np.float64) ** 2).sum(1).astype(np.float32)  # [ML]
        x2b = np.ascontiguousarray(np.broadcast_to(x2, (128, ML)))
        yc = y[c * ML : (c + 1) * ML]
        ybm = np.zeros((IW, ML), dtype=np.float32)
        ybm[1 + yc, np.arange(ML)] = 1.0
        in_maps.append(
            {
                "reft": reft,
                "xt": xt,
                "x2b": x2b,
                "r2t": r2t,
                "ind": ind,
                "yb": ybm,
            }
        )
    return in_maps


def run(x, x_ref, y, y_ref, trace=False, trace_kwargs=None):
    from concourse.bass_utils import run_bass_kernel_spmd

    nc = _get_nc()
    in_maps = _prep_inputs(x, x_ref, y, y_ref)
    res = run_bass_kernel_spmd(
        nc,
        in_maps,
        list(range(NCORES)),
        trace=trace,
        **(trace_kwargs or {}),
    )
    ld = np.concatenate([res.results[c]["out_ld"].reshape(-1) for c in range(NCORES)])
    loss = np.float32(-(ld.astype(np.float64).mean()))
    return loss, res


def kernel(x, x_ref, y, y_ref):
    loss, _ = run(x, x_ref, y, y_ref)
    return np.asarray(loss, dtype=np.float32)
